# revision 2
# baseline (speedup 1.0000x reference)
"""Trainium2 Bass kernel for 2-layer residual BiLSTM (B=256, T=512, D=U=256).

Strategy v4 (direction-split data parallel + on-core layer pipelining +
prefetched input projections):
  - Cores 0-3 run the FORWARD direction on batch quarters (64 rows each);
    cores 4-7 run BACKWARD on time-reversed inputs (same SPMD program --
    reversal happens host-side, outputs un-reversed on assembly). The fw and
    bw chains only meet at the final average, which the host computes.
  - Each core runs BOTH layers as two interleaved streams: layer 1 lags
    layer 0 by LAG steps and consumes h0 from an SBUF ring (no DRAM
    round-trip, no phase barrier).
  - "T-layout": gate/unit dims on partitions, batch on the free dim; gate
    column order [g,i,f,o] so tanh(g) is one ACT and sigmoid splits into
    s_if (feeds the cell update ASAP) + s_o (only needed by the final h
    mul, runs in the shadow of the DVE work).
  - TCP=2 gate-column PSUM groups with bufs=2 per layer: the input
    projection for group g+1 is emitted during group g's iterations, so
    the per-step critical cycle is only the 16 recurrent MMs + gate chain;
    the PSUM refill never stalls the pipe.  PSUM `start` is asserted once
    per 2KB bank (4 m-chunks at TCP=2), `stop` on each bank's last write.
  - Weights / x / h in fp16 (PE 1 cyc/row, FWL weight loads), PSUM/c fp32.
"""

import os

os.environ.setdefault("JAX_COMPILATION_CACHE_DIR", "/tmp/bilstm_jax_cache")

from contextlib import ExitStack

import numpy as np

# Problem shape (hardcoded per harness contract)
B, T, D, U = 256, 512, 256, 256
NCORES = 8
BS = B // (NCORES // 2)  # 64 batch rows per core (one direction per core)
G4 = 4 * U               # 1024 gate columns
NM = G4 // 128           # 8 m-chunks of gate columns
NK = U // 128            # 2 k-chunks of contraction dim
TCP = 2                  # steps per PSUM group (2 banks/tile, 2 bufs x 2 layers)
TCX = 32                 # steps per input ring chunk
LAG = 9                  # layer-1 lag
W = 16                   # h0 SBUF ring slots
OTCP = 4                 # output staging steps per DMA

# gate column permutation: original order [i f g o] -> ours [g i f o]
_GATE_PERM = np.r_[2 * U:3 * U, 0:U, U:2 * U, 3 * U:4 * U]

_BUILD_CACHE = {}


def _build(T_, dtype="fp16", steps=None, out_slots=None):
    """Build the SPMD Bass program (same program on all cores).

    steps/out_slots: timing-only variants -- run more (wrapped) or fewer
    scan steps and/or write outputs into a small ring to shrink the
    per-call output volume. Defaults (None) build the real kernel.
    """
    import concourse.bacc as bacc
    import concourse.bass as bass
    import concourse.mybir as mybir
    import concourse.tile as tile

    steps = T_ if steps is None else steps
    oslots = T_ if out_slots is None else out_slots
    ngrp = steps // TCP

    f32 = mybir.dt.float32
    wdt = {"fp32": f32, "bf16": mybir.dt.bfloat16, "fp16": mybir.dt.float16}[dtype]
    AF = mybir.ActivationFunctionType

    nc = bacc.Bacc("TRN2", target_bir_lowering=False, debug=False)

    xT = nc.dram_tensor("xT", [NK, 128, T_, BS], wdt, kind="ExternalInput")
    Wd = {}
    for l in (0, 1):
        for wch in "xh":
            Wd[l, wch] = nc.dram_tensor(
                f"W{wch}{l}", [NK, 128, G4], wdt, kind="ExternalInput"
            )
    out_t = nc.dram_tensor("out", [oslots, 128, NK, BS], f32, kind="ExternalOutput")

    with ExitStack() as ctx:
        tc = ctx.enter_context(tile.TileContext(nc))
        wpool = ctx.enter_context(tc.tile_pool(name="w", bufs=1))
        ring = ctx.enter_context(tc.tile_pool(name="ring", bufs=3))
        state = ctx.enter_context(tc.tile_pool(name="state", bufs=1))
        gates = ctx.enter_context(tc.tile_pool(name="gates", bufs=3))
        outp = ctx.enter_context(tc.tile_pool(name="outp", bufs=6))
        psum = ctx.enter_context(
            tc.tile_pool(name="psum", bufs=2, space=bass.MemorySpace.PSUM)
        )

        # --- load weights (both layers) into SBUF once ---
        wsb = {}
        for l in (0, 1):
            for wch in "xh":
                t = wpool.tile([128, NK, G4], wdt, tag=f"W{wch}{l}", name=f"W{wch}{l}sb")
                for k in range(NK):
                    nc.sync.dma_start(t[:, k, :], Wd[l, wch][k])
                wsb[l, wch] = t

        # persistent state
        h0r = state.tile([128, NK, W, BS], wdt, tag="h0r", name="h0r")
        c0 = state.tile([128, NK, BS], f32, tag="c0", name="c0")
        c1 = state.tile([128, NK, BS], f32, tag="c1", name="c1")
        h1 = [
            state.tile([128, NK, BS], wdt, tag=f"h1_{i}", name=f"h1_{i}")
            for i in range(4)
        ]
        nc.gpsimd.memset(h0r[:, :, W - 1, :], 0.0)
        nc.gpsimd.memset(c0[:], 0.0)
        nc.gpsimd.memset(c1[:], 0.0)
        nc.gpsimd.memset(h1[0][:], 0.0)

        # x ring: chunk c DMA'd half a chunk early (chunk 0 in prologue)
        ringt = {}

        def ring_dma(c):
            rt = ring.tile([128, NK, TCX, BS], wdt, tag="ring0", name=f"ring{c}")
            base = (c * TCX) % T_
            for k in range(NK):
                nc.sync.dma_start(rt[:, k, :, :], xT[k, :, base:base + TCX, :])
            ringt[c] = rt

        ring_dma(0)

        z0t = {}
        z1t = {}

        def xproj0(g, mlist):
            if g >= ngrp:
                return
            if g not in z0t:
                z0t[g] = psum.tile([128, NM, TCP, BS], f32, tag="z0", name=f"z0g{g}")
                z0t.pop(g - 2, None)
            zt = z0t[g]
            tbase = g * TCP
            rt = ringt[tbase // TCX]
            off = tbase % TCX
            for m in mlist:
                for k in range(NK):
                    nc.tensor.matmul(
                        zt[:, m, :, :],
                        wsb[0, "x"][:, k, m * 128:(m + 1) * 128],
                        rt[:, k, off:off + TCP, :],
                        # `start` clears the whole 2KB PSUM bank: assert it
                        # only on the first MM touching each bank (4 m-chunks
                        # per bank at TCP=2)
                        start=(k == 0 and m % 4 == 0),
                        stop=False,
                        skip_group_check=True,
                    )

        def xproj1(g, mlist):
            if g >= ngrp:
                return
            if g not in z1t:
                z1t[g] = psum.tile([128, NM, TCP, BS], f32, tag="z1", name=f"z1g{g}")
                z1t.pop(g - 2, None)
            zt = z1t[g]
            s = (g * TCP) % W
            for m in mlist:
                for k in range(NK):
                    nc.tensor.matmul(
                        zt[:, m, :, :],
                        wsb[1, "x"][:, k, m * 128:(m + 1) * 128],
                        h0r[:, k, s:s + TCP, :],
                        start=(k == 0 and m % 4 == 0),
                        stop=False,
                        skip_group_check=True,
                    )

        def rec(l, zt, j, hsrc, last):
            for m in range(NM):
                for k in range(NK):
                    nc.tensor.matmul(
                        zt[:, m, j, :],
                        wsb[l, "h"][:, k, m * 128:(m + 1) * 128],
                        hsrc[:, k, :],
                        start=False,
                        stop=(last and k == NK - 1 and m % 4 == 3),
                        skip_group_check=True,
                    )

        xproj0(0, range(NM))

        hidx1 = 0
        otile = None

        for r in range(steps + LAG):
            # ---------------- PE work ----------------
            if r < steps:
                t0 = r
                j0 = t0 % TCP
                if t0 % TCX == TCX // 2:
                    ring_dma(t0 // TCX + 1)
                g0 = t0 // TCP
                # prefetch next group's xproj, 4 m-chunks per iteration
                xproj0(g0 + 1, range(4 * j0, 4 * j0 + 4))
                rec(0, z0t[g0], j0, h0r[:, :, (t0 - 1) % W, :], last=(j0 == TCP - 1))

            if r >= LAG:
                t1 = r - LAG
                j1 = t1 % TCP
                g1 = t1 // TCP
                if g1 == 0 and j1 == 0:
                    xproj1(0, range(NM))
                xproj1(g1 + 1, range(4 * j1, 4 * j1 + 4))
                rec(1, z1t[g1], j1, h1[hidx1 % 4][:, :, :], last=(j1 == TCP - 1))

            # ---------------- gate chains ----------------
            tg0 = tg1 = None
            if r < steps:
                z = z0t[t0 // TCP]
                tg0 = gates.tile([128, NK, BS], wdt, tag="tg0")
                nc.scalar.activation(tg0[:], z[:, 0:NK, j0, :], AF.Tanh, bias=1.0)
                sif0 = gates.tile([128, 2 * NK, BS], wdt, tag="sif0")
                nc.scalar.activation(sif0[:], z[:, NK:3 * NK, j0, :], AF.Sigmoid,
                                     bias=1.0)
            if r >= LAG:
                z = z1t[t1 // TCP]
                tg1 = gates.tile([128, NK, BS], wdt, tag="tg1")
                nc.scalar.activation(tg1[:], z[:, 0:NK, j1, :], AF.Tanh, bias=1.0)
                sif1 = gates.tile([128, 2 * NK, BS], wdt, tag="sif1")
                nc.scalar.activation(sif1[:], z[:, NK:3 * NK, j1, :], AF.Sigmoid,
                                     bias=1.0)
            # s_o after both s_if's (only needed by the final muls)
            if tg0 is not None:
                so0 = gates.tile([128, NK, BS], wdt, tag="so0")
                nc.scalar.activation(so0[:], z0t[t0 // TCP][:, 3 * NK:NM, j0, :],
                                     AF.Sigmoid, bias=1.0)
            if tg1 is not None:
                so1 = gates.tile([128, NK, BS], wdt, tag="so1")
                nc.scalar.activation(so1[:], z1t[t1 // TCP][:, 3 * NK:NM, j1, :],
                                     AF.Sigmoid, bias=1.0)

            if tg0 is not None:
                ig0 = gates.tile([128, NK, BS], wdt, tag="ig0")
                nc.vector.tensor_mul(ig0[:], sif0[:, 0:NK, :], tg0[:])
                fc0 = gates.tile([128, NK, BS], f32, tag="fc0")
                nc.vector.tensor_mul(fc0[:], sif0[:, NK:2 * NK, :], c0[:])
                nc.vector.tensor_add(c0[:], ig0[:], fc0[:])
            if tg1 is not None:
                ig1 = gates.tile([128, NK, BS], wdt, tag="ig1")
                nc.vector.tensor_mul(ig1[:], sif1[:, 0:NK, :], tg1[:])
                fc1 = gates.tile([128, NK, BS], f32, tag="fc1")
                nc.vector.tensor_mul(fc1[:], sif1[:, NK:2 * NK, :], c1[:])
                nc.vector.tensor_add(c1[:], ig1[:], fc1[:])

            if tg0 is not None:
                th0 = gates.tile([128, NK, BS], wdt, tag="th0")
                nc.scalar.activation(th0[:], c0[:], AF.Tanh)
            if tg1 is not None:
                th1 = gates.tile([128, NK, BS], wdt, tag="th1")
                nc.scalar.activation(th1[:], c1[:], AF.Tanh)
            if tg0 is not None:
                nc.vector.tensor_mul(h0r[:, :, t0 % W, :], so0[:], th0[:])
            if tg1 is not None:
                hn = h1[(hidx1 + 1) % 4]
                nc.vector.tensor_mul(hn[:], so1[:], th1[:])
                # residual + output staging: accumulate OTCP steps per DMA
                jo = t1 % OTCP
                if jo == 0:
                    otile = outp.tile([128, OTCP, NK, BS], f32, tag="ot")
                nc.gpsimd.tensor_add(
                    otile[:, jo, :, :], hn[:], h0r[:, :, t1 % W, :]
                )
                if jo == OTCP - 1:
                    t1b = (t1 - (OTCP - 1)) % oslots
                    nc.sync.dma_start(
                        out_t[t1b:t1b + OTCP].rearrange("t p k b -> p t (k b)"),
                        otile.rearrange("p t k b -> p t (k b)"),
                    )
                hidx1 += 1

    nc.compile()
    return nc


def _prep_inputs(inputs, T_, dtype="fp16"):
    """Host-side shard + layout prep. Returns per-core input maps."""
    import ml_dtypes

    wdt = {"fp32": np.float32, "bf16": ml_dtypes.bfloat16, "fp16": np.float16}[dtype]

    x = np.asarray(inputs["x"], dtype=np.float32)

    wmaps = {}  # per direction
    for d, dd in (("f", "fw"), ("b", "bw")):
        m = {}
        for l in (0, 1):
            for wch, key in (("x", "Wx"), ("h", "Wh")):
                w = np.asarray(inputs[f"{dd}{l}_{key}"], dtype=np.float32)
                wp = w[:, _GATE_PERM].reshape(NK, 128, G4)
                m[f"W{wch}{l}"] = np.ascontiguousarray(wp).astype(wdt)
            bb = np.asarray(inputs[f"{dd}{l}_b"], dtype=np.float32)
            if not np.allclose(bb, 1.0, atol=0.0):
                raise NotImplementedError(
                    "kernel assumes bias == ones (keras bias_initializer='ones')"
                )
        wmaps[d] = m

    in_maps = []
    for ci in range(NCORES):
        d = "f" if ci < 4 else "b"
        q = ci % 4
        xs = x[q * BS:(q + 1) * BS, :T_, :]            # [BS, T_, D]
        if d == "b":
            xs = xs[:, ::-1, :]                        # time-reverse for bw
        xTc = np.ascontiguousarray(xs.transpose(2, 1, 0))  # [D, T_, BS]
        xTc = xTc.reshape(NK, 128, T_, BS).astype(wdt)
        m = {"xT": xTc}
        m.update(wmaps[d])
        in_maps.append(m)
    return in_maps


def _assemble(results, T_):
    out = np.empty((B, T_, U), dtype=np.float32)
    for q in range(4):
        af = results[q]["out"]          # [T_, 128, NK, BS] fw
        ab = results[q + 4]["out"]      # [T_, 128, NK, BS] bw (reversed time)
        # out[b, t, k*128 + p] = arr[t, p, k, b]
        f = af.transpose(3, 0, 2, 1).reshape(BS, T_, U)
        bwd = ab[::-1].transpose(3, 0, 2, 1).reshape(BS, T_, U)
        out[q * BS:(q + 1) * BS] = (f + bwd) * 0.5
    return out


def _setup_jax_cache():
    try:
        import jax

        jax.config.update("jax_compilation_cache_dir",
                          os.environ["JAX_COMPILATION_CACHE_DIR"])
        jax.config.update("jax_persistent_cache_min_compile_time_secs", 1.0)
        jax.config.update("jax_persistent_cache_min_entry_size_bytes", 0)
    except Exception:
        pass


def kernel(**inputs) -> np.ndarray:
    _setup_jax_cache()
    from concourse.bass_utils import run_bass_kernel_spmd

    dtype = "fp16"
    key = (T, dtype)
    if key not in _BUILD_CACHE:
        _BUILD_CACHE[key] = _build(T, dtype)
    nc = _BUILD_CACHE[key]

    in_maps = _prep_inputs(inputs, T, dtype)
    res = run_bass_kernel_spmd(nc, in_maps, core_ids=list(range(NCORES)))
    return _assemble(res.results, T)
